# revision 15
# baseline (speedup 1.0000x reference)
"""Trainium2 Bass kernel for AttentionConvFull (local 5x5 window attention
with per-channel softmax, grouped 1x1 conv projections).

Sharding: 8 cores = batch(4) x H-halves(2). Each core gets a 32-row halo'd,
zero-padded slice of x, pre-transposed on host to channel-major [256, 32*60].
No collectives needed.

v4 dataflow (vs the STT baseline): the fused scalar_tensor_tensor ran at
1x DVE mode (1.04 ns/elem measured). Split into:
  kr_j = k_j + rel_j   -- tensor_scalar per tap: 4x DVE mode (0.26 ns/elem)
                          or ACT identity+bias (split to balance engines)
  t    = kr (.) q      -- one grouped 2x tensor_tensor per dj-group,
                          q broadcast over taps via stride-0 AP dim
  e    = exp(t)        -- ACT, grouped
  w    = e (.) v_win   -- 2x tensor_tensor (overlapping strided v view)
  den += e_j, num += w_j on PE via identity matmuls (measured: overlapped
  back-to-back matmuls stream at 0.417 ns/col when the p-state stays hot).
Epilogue: den/num cast PSUM->SBUF bf16, DMA out; final num/den on host.
"""

import numpy as np
import ml_dtypes

import concourse.bass as bass
import concourse.tile as tile
from concourse import bacc, mybir
from concourse.ap import AP
from concourse.bass_utils import run_bass_kernel_spmd

F32 = mybir.dt.float32
BF16 = mybir.dt.bfloat16

K = 5
G = 8
B, H, W, C = 4, 56, 56, 256
Cg = C // G            # 32
P = K // 2             # 2
HS = H // 2            # 28 output rows per shard
MR = HS + 2 * P        # 32 map rows
MC = W + 2 * P         # 60 map cols
SP = MR * MC           # 1920 map spatial
OP = HS * W            # 1568 output spatial per shard
NCH = 2                # channel chunks of 128 partitions
NCORES = 8
HALF = OP // 2         # 784: PSUM accumulator tile size


def _dedup_ldweights(nc):
    """Remove redundant PE weight reloads: consecutive InstLdweights that
    load the same stationary operand with no sync info."""
    removed = 0
    for blk in nc.main_func.blocks:
        last_sig = None
        keep = []
        for inst in blk.instructions:
            if isinstance(inst, mybir.InstLdweights):
                sig = " ".join(a.concise() for a in inst.ins)
                si = inst.sync_info
                clean = si is None or (
                    len(si.on_wait) == 0 and len(si.on_update) == 0
                )
                if sig == last_sig and clean:
                    removed += 1
                    continue
                last_sig = sig
            elif isinstance(inst, mybir.InstMatmult):
                if len(inst.ins) > 1:
                    wsig = inst.ins[1].concise()
                    if wsig != last_sig:
                        last_sig = wsig
            keep.append(inst)
        blk.instructions[:] = keep
    return removed


def _strided_view(base, extra_offset, dims):
    """Custom strided/broadcast view of a 2D [128, N] tile AP.
    dims: list of (stride, num) free dims, outer->inner."""
    pairs = list(base.ap)
    pstride, pnum = pairs[0]
    return AP(
        base.tensor,
        base.offset + extra_offset,
        [[pstride, pnum]] + [[s, n] for s, n in dims],
    )


def build_nc():
    nc = bacc.Bacc(
        "TRN2", target_bir_lowering=False, debug=False, num_devices=NCORES
    )

    xt_d = nc.dram_tensor("xt", [NCH, 128, SP], BF16, kind="ExternalInput").ap()
    # weights batched into single DMAs: [wk | wq | wv | ident] bf16 and
    # [rel | qe] f32 (DMA issue on the sync queue costs ~600ns each)
    wall_d = nc.dram_tensor(
        "wall", [NCH, 128, 128 * 4], BF16, kind="ExternalInput"
    ).ap()
    rq_d = nc.dram_tensor(
        "rqb", [NCH, 128, K * K + 1], F32, kind="ExternalInput"
    ).ap()
    den_d = nc.dram_tensor("dd", [NCH, 128, OP], BF16, kind="ExternalOutput").ap()
    num_d = nc.dram_tensor("nd", [NCH, 128, OP], BF16, kind="ExternalOutput").ap()

    with tile.TileContext(nc) as tc:
        with (
            tc.tile_pool(name="weights", bufs=2) as wpool,
            tc.tile_pool(name="xin", bufs=2) as xpool,
            tc.tile_pool(name="maps", bufs=2) as mpool,
            tc.tile_pool(name="krgrp", bufs=2) as krpool,
            tc.tile_pool(name="tgrp", bufs=2) as tpool,
            tc.tile_pool(name="egrp", bufs=2) as epool,
            tc.tile_pool(name="wgrp", bufs=3) as wgpool,
            tc.tile_pool(name="epi", bufs=2) as opool,
            tc.tile_pool(name="pacc", bufs=4, space=bass.MemorySpace.PSUM) as apsum,
        ):
            kmaps, komaps, vmaps, vomaps, qflats, rels = (
                [None, None] for _ in range(6)
            )
            wts, qes, xsbs = [None, None], [None, None], [None, None]
            idents = [None, None]

            def emit_inputs(c):
                x_sb = xpool.tile([128, SP], BF16, tag="x", name=f"x{c}")
                nc.sync.dma_start(x_sb[:, : SP // 2], xt_d[c][:, : SP // 2])
                nc.sync.dma_start(x_sb[:, SP // 2 :], xt_d[c][:, SP // 2 :])
                xsbs[c] = x_sb
                wall = wpool.tile([128, 128 * 4], BF16, tag="wall", name=f"wall{c}")
                nc.sync.dma_start(wall[:], wall_d[c])
                wts[c] = {
                    "wk": wall[:, 0:128],
                    "wq": wall[:, 128:256],
                    "wv": wall[:, 256:384],
                }
                idents[c] = wall[:, 384:512]
                rq_sb = wpool.tile([128, K * K + 1], F32, tag="rq", name=f"rq{c}")
                nc.sync.dma_start(rq_sb[:], rq_d[c])
                rels[c] = rq_sb
                qes[c] = rq_sb[:, K * K : K * K + 1]

            def emit_proj(c):
                x_sb = xsbs[c]
                k_bf = mpool.tile([128, SP], BF16, tag="k", name=f"k{c}")
                v_bf = mpool.tile([128, SP], BF16, tag="v", name=f"v{c}")
                qf = mpool.tile([128, OP], BF16, tag="qf", name=f"qf{c}")
                NS = 2
                SL = SP // NS  # 960 (16 map rows per slice)
                # k first (unblocks the DVE j-loop), then q, then v.
                for nm in ("wk", "wq", "wv"):
                    for s in range(NS):
                        lo = s * SL
                        rhs = x_sb[:, lo : lo + SL]
                        ps = apsum.tile(
                            [128, SL], F32, tag="acc", name=f"pp{c}{s}{nm}"
                        )
                        for mlo, mn in ((0, 512), (512, SL - 512)):
                            nc.tensor.matmul(
                                ps[:, mlo : mlo + mn],
                                wts[c][nm],
                                rhs[:, mlo : mlo + mn],
                                start=True,
                                stop=True,
                            )
                        if nm == "wq":
                            # interior rows/cols of this 16-row band into
                            # flat q, fusing the q_emb per-partition bias
                            r0 = max(P, 16 * s)
                            r1 = min(MR - P, 16 * (s + 1))
                            src = ps[:].rearrange("p (h w) -> p h w", h=16)[
                                :, r0 - 16 * s : r1 - 16 * s, P : P + W
                            ]
                            dst = qf[:].rearrange("p (h w) -> p h w", h=HS)[
                                :, r0 - P : r1 - P, :
                            ]
                            nc.scalar.activation(
                                dst,
                                src,
                                mybir.ActivationFunctionType.Identity,
                                bias=qes[c],
                            )
                        else:
                            # k casts split across ACT/DVE so the map is
                            # ready fastest (it gates the whole j-loop);
                            # v casts on ACT (needed later).
                            dst_map = k_bf if nm == "wk" else v_bf
                            if nm == "wk" and s == 1:
                                nc.vector.tensor_copy(
                                    dst_map[:, lo : lo + SL], ps[:]
                                )
                            else:
                                nc.scalar.copy(dst_map[:, lo : lo + SL], ps[:])

                # 1-elem-shifted copies so odd window columns keep 4B align
                k_od = mpool.tile([128, SP], BF16, tag="ko", name=f"ko{c}")
                v_od = mpool.tile([128, SP], BF16, tag="vo", name=f"vo{c}")
                nc.sync.dma_start(k_od[:, : SP - 1], k_bf[:, 1:])
                nc.sync.dma_start(v_od[:, : SP - 1], v_bf[:, 1:])
                kmaps[c], komaps[c] = k_bf, k_od
                vmaps[c], vomaps[c] = v_bf, v_od
                qflats[c] = qf

            def emit_group(c, dj, den, num, di0=0, ndi=K, nact=0):
                dje = dj - (dj % 2)
                kc = kmaps[c] if dj % 2 == 0 else komaps[c]
                vc = vmaps[c] if dj % 2 == 0 else vomaps[c]
                nm = f"{c}{dj}{di0}"
                kr = krpool.tile([128, ndi * OP], BF16, tag="kr", name=f"kr{nm}")
                tg = tpool.tile([128, ndi * OP], BF16, tag="t", name=f"t{nm}")
                eg = epool.tile([128, ndi * OP], BF16, tag="e", name=f"e{nm}")
                wg = wgpool.tile([128, ndi * OP], BF16, tag="w", name=f"w{nm}")

                k3 = kc[:].rearrange("p (h w) -> p h w", h=MR)
                kr4 = kr[:].rearrange("p (j h w) -> p j h w", j=ndi, h=HS)
                # kr_j = k_j + rel_j: per-tap add of a per-partition scalar.
                # DVE tensor_scalar runs at 4x; ACT identity+bias costs ~1
                # elem/cycle. Split taps to balance engine load.
                n_act_rel = nact
                for idx in range(ndi):
                    di = di0 + idx
                    j = di * K + dj
                    kv = k3[:, di : di + HS, dje : dje + W]
                    if idx < n_act_rel:
                        nc.scalar.activation(
                            kr4[:, idx],
                            kv,
                            mybir.ActivationFunctionType.Identity,
                            bias=rels[c][:, j : j + 1],
                        )
                    else:
                        nc.vector.tensor_scalar(
                            kr4[:, idx],
                            kv,
                            rels[c][:, j : j + 1],
                            None,
                            mybir.AluOpType.add,
                        )

                # t = kr (.) q grouped 2x tensor_tensor; q broadcast over the
                # tap dim via a stride-0 AP dim. First group: per-tap, so the
                # chain to the first PE matmul is short (pipeline fill).
                tg3 = tg[:].rearrange("p (j n) -> p j n", j=ndi)
                kr3 = kr[:].rearrange("p (j n) -> p j n", j=ndi)
                first = c == 0 and dj == 0
                eg4 = eg[:].rearrange("p (j h w) -> p j h w", j=ndi, h=HS)
                wg4 = wg[:].rearrange("p (j h w) -> p j h w", j=ndi, h=HS)
                if first:
                    for sl in ((0, 1), (1, 3), (3, 5)):
                        a, b = sl
                        qb = _strided_view(
                            qflats[c][:], 0, [(0, b - a), (1, OP)]
                        )
                        nc.vector.tensor_tensor(
                            tg3[:, a:b], kr3[:, a:b], qb, mybir.AluOpType.mult
                        )
                        nc.scalar.activation(
                            eg[:, a * OP : b * OP],
                            tg[:, a * OP : b * OP],
                            mybir.ActivationFunctionType.Exp,
                        )
                        vwin = _strided_view(
                            vc[:],
                            dje + (di0 + a) * MC,
                            [(MC, b - a), (MC, HS), (1, W)],
                        )
                        nc.vector.tensor_tensor(
                            wg4[:, a:b], eg4[:, a:b], vwin, mybir.AluOpType.mult
                        )
                else:
                    qb = _strided_view(qflats[c][:], 0, [(0, ndi), (1, OP)])
                    nc.vector.tensor_tensor(
                        tg3, kr3, qb, mybir.AluOpType.mult
                    )
                    nc.scalar.activation(
                        eg[:], tg[:], mybir.ActivationFunctionType.Exp
                    )
                    vwin = _strided_view(
                        vc[:], dje + di0 * MC, [(MC, ndi), (MC, HS), (1, W)]
                    )
                    nc.vector.tensor_tensor(
                        wg4, eg4, vwin, mybir.AluOpType.mult
                    )

                eg3 = eg[:].rearrange("p (j n) -> p j n", j=ndi)
                wg3 = wg[:].rearrange("p (j n) -> p j n", j=ndi)
                # per-tap matmuls (ISA caps one matmul at 512 out elems);
                # alternate PSUM regions so consecutive mms hit different
                # banks: h0-512, h1-512, h0-272, h1-272 per tap.
                for acc, src3 in ((den, eg3), (num, wg3)):
                    for idx in range(ndi):
                        di = di0 + idx
                        for lo, n in ((0, 512), (512, HALF - 512)):
                            for h in range(2):
                                base = h * HALF
                                nc.tensor.matmul(
                                    acc[h][:, lo : lo + n],
                                    idents[c],
                                    src3[:, idx, base + lo : base + lo + n],
                                    start=dj == 0 and di == 0,
                                    stop=dj == K - 1 and di == K - 1,
                                )

            def emit_epilogue(c, den, num):
                den_sb = opool.tile([128, OP], BF16, tag="osb", name=f"dsb{c}")
                num_sb = opool.tile([128, OP], BF16, tag="osb", name=f"nsb{c}")
                # split the PSUM->SBUF drain across both engines so the
                # PSUM banks free up fast (chunk 1's accumulators wait on
                # chunk 0's drain).
                for h in range(2):
                    base = h * HALF
                    if h == 0:
                        nc.vector.tensor_copy(
                            den_sb[:, base : base + HALF], den[h][:]
                        )
                        nc.scalar.copy(num_sb[:, base : base + HALF], num[h][:])
                    else:
                        nc.scalar.copy(
                            den_sb[:, base : base + HALF], den[h][:]
                        )
                        nc.vector.tensor_copy(
                            num_sb[:, base : base + HALF], num[h][:]
                        )
                nc.sync.dma_start(den_d[c], den_sb[:])
                nc.sync.dma_start(num_d[c], num_sb[:])

            # ---- emission schedule ----
            emit_inputs(0)
            emit_proj(0)
            emit_inputs(1)
            emit_proj(1)
            accs = []
            for c in range(NCH):
                den = [
                    apsum.tile([128, HALF], F32, tag="acc", name=f"den{c}{h}")
                    for h in range(2)
                ]
                num = [
                    apsum.tile([128, HALF], F32, tag="acc", name=f"num{c}{h}")
                    for h in range(2)
                ]
                accs.append((den, num))
                for dj in range(K):
                    nact = 2 - (dj % 2)
                    if c == 1 and dj == K - 1:
                        # split the final group so the tail drain runs on
                        # smaller quanta
                        emit_group(c, dj, den, num, 0, 3, nact)
                        emit_group(c, dj, den, num, 3, 1)
                        emit_group(c, dj, den, num, 4, 1)
                    else:
                        emit_group(c, dj, den, num, 0, K, nact)
                emit_epilogue(c, den, num)

    nc.compile()
    _dedup_ldweights(nc)
    return nc


def _block_diag_weights(w):
    """w: (G, Cg_out, Cg_in) -> lhsT layout [NCH, 128, 128] where
    lhsT[c, ci, co] = w[g, co%32, ci%32] for matching 32-blocks."""
    out = np.zeros((NCH, 128, 128), np.float32)
    for c in range(NCH):
        for g4 in range(4):
            g = c * 4 + g4
            blk = w[g]  # (Cg_out, Cg_in)
            out[c, g4 * 32 : (g4 + 1) * 32, g4 * 32 : (g4 + 1) * 32] = blk.T
    return out


_NC_CACHE = {}


def _make_in_maps(inputs):
    x = np.asarray(inputs["x"], np.float32)
    wq = np.asarray(inputs["wq"], np.float32)
    wk = np.asarray(inputs["wk"], np.float32)
    wv = np.asarray(inputs["wv"], np.float32)
    rel_emb = np.asarray(inputs["rel_emb"], np.float32)
    q_emb = np.asarray(inputs["q_emb"], np.float32)

    bf = ml_dtypes.bfloat16
    wqb = _block_diag_weights(wq)
    wkb = _block_diag_weights(wk)
    wvb = _block_diag_weights(wv)
    idn = np.broadcast_to(np.eye(128, dtype=np.float32), (NCH, 128, 128))
    wall = np.ascontiguousarray(
        np.concatenate([wkb, wqb, wvb, idn], axis=2)
    ).astype(bf)
    relb = rel_emb.reshape(G, Cg, K * K).reshape(NCH, 128, K * K)
    qeb = q_emb.reshape(NCH, 128, 1)
    rqb = np.ascontiguousarray(np.concatenate([relb, qeb], axis=2))

    xp = np.pad(x, ((0, 0), (P, P), (P, P), (0, 0)))  # (B, 60, 60, C)

    in_maps = []
    for core in range(NCORES):
        b, half = divmod(core, 2)
        sh = xp[b, HS * half : HS * half + MR]         # (32, 60, C)
        xt = np.ascontiguousarray(sh.reshape(SP, C).T).reshape(NCH, 128, SP)
        in_maps.append(
            {
                "xt": xt.astype(bf),
                "wall": wall,
                "rqb": rqb,
            }
        )
    return in_maps


def kernel(**inputs):
    in_maps = _make_in_maps(inputs)

    if "nc" not in _NC_CACHE:
        _NC_CACHE["nc"] = build_nc()
    nc = _NC_CACHE["nc"]

    res = run_bass_kernel_spmd(nc, in_maps, core_ids=list(range(NCORES)))

    out = np.empty((B, H, W, C), np.float32)
    for core in range(NCORES):
        b, half = divmod(core, 2)
        den = res.results[core]["dd"].astype(np.float32).reshape(C, HS, W)
        num = res.results[core]["nd"].astype(np.float32).reshape(C, HS, W)
        o = num / den
        out[b, HS * half : HS * half + HS] = o.transpose(1, 2, 0)
    return out


# revision 16
# speedup vs baseline: 1.0362x; 1.0362x over previous
"""Trainium2 Bass kernel for AttentionConvFull (local 5x5 window attention
with per-channel softmax, grouped 1x1 conv projections).

Sharding: 8 cores = batch(4) x H-halves(2). Each core gets a 32-row halo'd,
zero-padded slice of x, pre-transposed on host to channel-major [256, 32*60].
No collectives needed.

v4 dataflow (vs the STT baseline): the fused scalar_tensor_tensor ran at
1x DVE mode (1.04 ns/elem measured). Split into:
  kr_j = k_j + rel_j   -- tensor_scalar per tap: 4x DVE mode (0.26 ns/elem)
                          or ACT identity+bias (split to balance engines)
  t    = kr (.) q      -- one grouped 2x tensor_tensor per dj-group,
                          q broadcast over taps via stride-0 AP dim
  e    = exp(t)        -- ACT, grouped
  w    = e (.) v_win   -- 2x tensor_tensor (overlapping strided v view)
  den += e_j, num += w_j on PE via identity matmuls (measured: overlapped
  back-to-back matmuls stream at 0.417 ns/col when the p-state stays hot).
Epilogue: den/num cast PSUM->SBUF bf16, DMA out; final num/den on host.
"""

import numpy as np
import ml_dtypes

import concourse.bass as bass
import concourse.tile as tile
from concourse import bacc, mybir
from concourse.ap import AP
from concourse.bass_utils import run_bass_kernel_spmd

F32 = mybir.dt.float32
BF16 = mybir.dt.bfloat16

K = 5
G = 8
B, H, W, C = 4, 56, 56, 256
Cg = C // G            # 32
P = K // 2             # 2
HS = H // 2            # 28 output rows per shard
MR = HS + 2 * P        # 32 map rows
MC = W + 2 * P         # 60 map cols
SP = MR * MC           # 1920 map spatial
OP = HS * W            # 1568 output spatial per shard
NCH = 2                # channel chunks of 128 partitions
NCORES = 8
HALF = OP // 2         # 784: PSUM accumulator tile size


def _dedup_ldweights(nc):
    """Remove redundant PE weight reloads: consecutive InstLdweights that
    load the same stationary operand with no sync info."""
    removed = 0
    for blk in nc.main_func.blocks:
        last_sig = None
        keep = []
        for inst in blk.instructions:
            if isinstance(inst, mybir.InstLdweights):
                sig = " ".join(a.concise() for a in inst.ins)
                si = inst.sync_info
                clean = si is None or (
                    len(si.on_wait) == 0 and len(si.on_update) == 0
                )
                if sig == last_sig and clean:
                    removed += 1
                    continue
                last_sig = sig
            elif isinstance(inst, mybir.InstMatmult):
                if len(inst.ins) > 1:
                    wsig = inst.ins[1].concise()
                    if wsig != last_sig:
                        last_sig = wsig
            keep.append(inst)
        blk.instructions[:] = keep
    return removed


def _strided_view(base, extra_offset, dims):
    """Custom strided/broadcast view of a 2D [128, N] tile AP.
    dims: list of (stride, num) free dims, outer->inner."""
    pairs = list(base.ap)
    pstride, pnum = pairs[0]
    return AP(
        base.tensor,
        base.offset + extra_offset,
        [[pstride, pnum]] + [[s, n] for s, n in dims],
    )


def build_nc():
    nc = bacc.Bacc(
        "TRN2", target_bir_lowering=False, debug=False, num_devices=NCORES
    )

    xt_d = nc.dram_tensor("xt", [NCH, 128, SP], BF16, kind="ExternalInput").ap()
    # weights batched into single DMAs: [wk | wq | wv | ident] bf16 and
    # [rel | qe] f32 (DMA issue on the sync queue costs ~600ns each)
    wall_d = nc.dram_tensor(
        "wall", [NCH, 128, 128 * 4], BF16, kind="ExternalInput"
    ).ap()
    rq_d = nc.dram_tensor(
        "rqb", [NCH, 128, K * K + 1], F32, kind="ExternalInput"
    ).ap()
    den_d = nc.dram_tensor("dd", [NCH, 128, OP], BF16, kind="ExternalOutput").ap()
    num_d = nc.dram_tensor("nd", [NCH, 128, OP], BF16, kind="ExternalOutput").ap()

    with tile.TileContext(nc) as tc:
        with (
            tc.tile_pool(name="weights", bufs=2) as wpool,
            tc.tile_pool(name="xin", bufs=2) as xpool,
            tc.tile_pool(name="maps", bufs=2) as mpool,
            tc.tile_pool(name="krgrp", bufs=2) as krpool,
            tc.tile_pool(name="tgrp", bufs=2) as tpool,
            tc.tile_pool(name="egrp", bufs=2) as epool,
            tc.tile_pool(name="wgrp", bufs=3) as wgpool,
            tc.tile_pool(name="epi", bufs=2) as opool,
            tc.tile_pool(name="pacc", bufs=4, space=bass.MemorySpace.PSUM) as apsum,
        ):
            kmaps, komaps, vmaps, vomaps, qflats, rels = (
                [None, None] for _ in range(6)
            )
            wts, qes, xsbs = [None, None], [None, None], [None, None]
            idents = [None, None]

            def emit_inputs(c):
                x_sb = xpool.tile([128, SP], BF16, tag="x", name=f"x{c}")
                nc.sync.dma_start(x_sb[:], xt_d[c])
                xsbs[c] = x_sb
                wall = wpool.tile([128, 128 * 4], BF16, tag="wall", name=f"wall{c}")
                nc.sync.dma_start(wall[:], wall_d[c])
                wts[c] = {
                    "wk": wall[:, 0:128],
                    "wq": wall[:, 128:256],
                    "wv": wall[:, 256:384],
                }
                idents[c] = wall[:, 384:512]
                rq_sb = wpool.tile([128, K * K + 1], F32, tag="rq", name=f"rq{c}")
                nc.sync.dma_start(rq_sb[:], rq_d[c])
                rels[c] = rq_sb
                qes[c] = rq_sb[:, K * K : K * K + 1]

            def emit_proj(c):
                x_sb = xsbs[c]
                k_bf = mpool.tile([128, SP], BF16, tag="k", name=f"k{c}")
                v_bf = mpool.tile([128, SP], BF16, tag="v", name=f"v{c}")
                qf = mpool.tile([128, OP], BF16, tag="qf", name=f"qf{c}")
                NS = 2
                SL = SP // NS  # 960 (16 map rows per slice)
                # k first (unblocks the DVE j-loop), then q, then v.
                for nm in ("wk", "wq", "wv"):
                    for s in range(NS):
                        lo = s * SL
                        rhs = x_sb[:, lo : lo + SL]
                        ps = apsum.tile(
                            [128, SL], F32, tag="acc", name=f"pp{c}{s}{nm}"
                        )
                        for mlo, mn in ((0, 512), (512, SL - 512)):
                            nc.tensor.matmul(
                                ps[:, mlo : mlo + mn],
                                wts[c][nm],
                                rhs[:, mlo : mlo + mn],
                                start=True,
                                stop=True,
                            )
                        if nm == "wq":
                            # interior rows/cols of this 16-row band into
                            # flat q, fusing the q_emb per-partition bias
                            r0 = max(P, 16 * s)
                            r1 = min(MR - P, 16 * (s + 1))
                            src = ps[:].rearrange("p (h w) -> p h w", h=16)[
                                :, r0 - 16 * s : r1 - 16 * s, P : P + W
                            ]
                            dst = qf[:].rearrange("p (h w) -> p h w", h=HS)[
                                :, r0 - P : r1 - P, :
                            ]
                            nc.scalar.activation(
                                dst,
                                src,
                                mybir.ActivationFunctionType.Identity,
                                bias=qes[c],
                            )
                        else:
                            # k casts split across ACT/DVE so the map is
                            # ready fastest (it gates the whole j-loop);
                            # v casts on ACT (needed later).
                            dst_map = k_bf if nm == "wk" else v_bf
                            if nm == "wk" and s == 1:
                                nc.vector.tensor_copy(
                                    dst_map[:, lo : lo + SL], ps[:]
                                )
                            else:
                                nc.scalar.copy(dst_map[:, lo : lo + SL], ps[:])

                # 1-elem-shifted copies so odd window columns keep 4B align
                k_od = mpool.tile([128, SP], BF16, tag="ko", name=f"ko{c}")
                v_od = mpool.tile([128, SP], BF16, tag="vo", name=f"vo{c}")
                nc.sync.dma_start(k_od[:, : SP - 1], k_bf[:, 1:])
                nc.sync.dma_start(v_od[:, : SP - 1], v_bf[:, 1:])
                kmaps[c], komaps[c] = k_bf, k_od
                vmaps[c], vomaps[c] = v_bf, v_od
                qflats[c] = qf

            def emit_group(c, dj, den, num, di0=0, ndi=K, nact=0):
                dje = dj - (dj % 2)
                kc = kmaps[c] if dj % 2 == 0 else komaps[c]
                vc = vmaps[c] if dj % 2 == 0 else vomaps[c]
                nm = f"{c}{dj}{di0}"
                kr = krpool.tile([128, ndi * OP], BF16, tag="kr", name=f"kr{nm}")
                tg = tpool.tile([128, ndi * OP], BF16, tag="t", name=f"t{nm}")
                eg = epool.tile([128, ndi * OP], BF16, tag="e", name=f"e{nm}")
                wg = wgpool.tile([128, ndi * OP], BF16, tag="w", name=f"w{nm}")

                k3 = kc[:].rearrange("p (h w) -> p h w", h=MR)
                kr4 = kr[:].rearrange("p (j h w) -> p j h w", j=ndi, h=HS)
                # kr_j = k_j + rel_j: per-tap add of a per-partition scalar.
                # DVE tensor_scalar runs at 4x; ACT identity+bias costs ~1
                # elem/cycle. Split taps to balance engine load.
                n_act_rel = nact
                for idx in range(ndi):
                    di = di0 + idx
                    j = di * K + dj
                    kv = k3[:, di : di + HS, dje : dje + W]
                    if idx < n_act_rel:
                        nc.scalar.activation(
                            kr4[:, idx],
                            kv,
                            mybir.ActivationFunctionType.Identity,
                            bias=rels[c][:, j : j + 1],
                        )
                    else:
                        nc.vector.tensor_scalar(
                            kr4[:, idx],
                            kv,
                            rels[c][:, j : j + 1],
                            None,
                            mybir.AluOpType.add,
                        )

                # t = kr (.) q grouped 2x tensor_tensor; q broadcast over the
                # tap dim via a stride-0 AP dim. First group: per-tap, so the
                # chain to the first PE matmul is short (pipeline fill).
                tg3 = tg[:].rearrange("p (j n) -> p j n", j=ndi)
                kr3 = kr[:].rearrange("p (j n) -> p j n", j=ndi)
                first = c == 0 and dj == 0
                eg4 = eg[:].rearrange("p (j h w) -> p j h w", j=ndi, h=HS)
                wg4 = wg[:].rearrange("p (j h w) -> p j h w", j=ndi, h=HS)
                if first:
                    for sl in ((0, 1), (1, 3), (3, 5)):
                        a, b = sl
                        qb = _strided_view(
                            qflats[c][:], 0, [(0, b - a), (1, OP)]
                        )
                        nc.vector.tensor_tensor(
                            tg3[:, a:b], kr3[:, a:b], qb, mybir.AluOpType.mult
                        )
                        nc.scalar.activation(
                            eg[:, a * OP : b * OP],
                            tg[:, a * OP : b * OP],
                            mybir.ActivationFunctionType.Exp,
                        )
                        vwin = _strided_view(
                            vc[:],
                            dje + (di0 + a) * MC,
                            [(MC, b - a), (MC, HS), (1, W)],
                        )
                        nc.vector.tensor_tensor(
                            wg4[:, a:b], eg4[:, a:b], vwin, mybir.AluOpType.mult
                        )
                else:
                    qb = _strided_view(qflats[c][:], 0, [(0, ndi), (1, OP)])
                    nc.vector.tensor_tensor(
                        tg3, kr3, qb, mybir.AluOpType.mult
                    )
                    nc.scalar.activation(
                        eg[:], tg[:], mybir.ActivationFunctionType.Exp
                    )
                    vwin = _strided_view(
                        vc[:], dje + di0 * MC, [(MC, ndi), (MC, HS), (1, W)]
                    )
                    nc.vector.tensor_tensor(
                        wg4, eg4, vwin, mybir.AluOpType.mult
                    )

                eg3 = eg[:].rearrange("p (j n) -> p j n", j=ndi)
                wg3 = wg[:].rearrange("p (j n) -> p j n", j=ndi)
                # per-tap matmuls (ISA caps one matmul at 512 out elems);
                # alternate PSUM regions so consecutive mms hit different
                # banks: h0-512, h1-512, h0-272, h1-272 per tap.
                for acc, src3 in ((den, eg3), (num, wg3)):
                    for idx in range(ndi):
                        di = di0 + idx
                        for lo, n in ((0, 512), (512, HALF - 512)):
                            for h in range(2):
                                base = h * HALF
                                nc.tensor.matmul(
                                    acc[h][:, lo : lo + n],
                                    idents[c],
                                    src3[:, idx, base + lo : base + lo + n],
                                    start=dj == 0 and di == 0,
                                    stop=dj == K - 1 and di == K - 1,
                                )

            def emit_epilogue(c, den, num):
                den_sb = opool.tile([128, OP], BF16, tag="osb", name=f"dsb{c}")
                num_sb = opool.tile([128, OP], BF16, tag="osb", name=f"nsb{c}")
                # split the PSUM->SBUF drain across both engines so the
                # PSUM banks free up fast (chunk 1's accumulators wait on
                # chunk 0's drain).
                for h in range(2):
                    base = h * HALF
                    if h == 0:
                        nc.vector.tensor_copy(
                            den_sb[:, base : base + HALF], den[h][:]
                        )
                        nc.scalar.copy(num_sb[:, base : base + HALF], num[h][:])
                    else:
                        nc.scalar.copy(
                            den_sb[:, base : base + HALF], den[h][:]
                        )
                        nc.vector.tensor_copy(
                            num_sb[:, base : base + HALF], num[h][:]
                        )
                nc.sync.dma_start(den_d[c], den_sb[:])
                nc.sync.dma_start(num_d[c], num_sb[:])

            # ---- emission schedule ----
            emit_inputs(0)
            emit_proj(0)
            emit_inputs(1)
            emit_proj(1)
            accs = []
            for c in range(NCH):
                den = [
                    apsum.tile([128, HALF], F32, tag="acc", name=f"den{c}{h}")
                    for h in range(2)
                ]
                num = [
                    apsum.tile([128, HALF], F32, tag="acc", name=f"num{c}{h}")
                    for h in range(2)
                ]
                accs.append((den, num))
                for dj in range(K):
                    nact = 2 - (dj % 2)
                    if c == 1 and dj == K - 1:
                        # split the final group so the tail drain runs on
                        # smaller quanta
                        emit_group(c, dj, den, num, 0, 3)
                        emit_group(c, dj, den, num, 3, 1)
                        emit_group(c, dj, den, num, 4, 1)
                    else:
                        emit_group(c, dj, den, num, 0, K, nact)
                emit_epilogue(c, den, num)

    nc.compile()
    _dedup_ldweights(nc)
    return nc


def _block_diag_weights(w):
    """w: (G, Cg_out, Cg_in) -> lhsT layout [NCH, 128, 128] where
    lhsT[c, ci, co] = w[g, co%32, ci%32] for matching 32-blocks."""
    out = np.zeros((NCH, 128, 128), np.float32)
    for c in range(NCH):
        for g4 in range(4):
            g = c * 4 + g4
            blk = w[g]  # (Cg_out, Cg_in)
            out[c, g4 * 32 : (g4 + 1) * 32, g4 * 32 : (g4 + 1) * 32] = blk.T
    return out


_NC_CACHE = {}


def _make_in_maps(inputs):
    x = np.asarray(inputs["x"], np.float32)
    wq = np.asarray(inputs["wq"], np.float32)
    wk = np.asarray(inputs["wk"], np.float32)
    wv = np.asarray(inputs["wv"], np.float32)
    rel_emb = np.asarray(inputs["rel_emb"], np.float32)
    q_emb = np.asarray(inputs["q_emb"], np.float32)

    bf = ml_dtypes.bfloat16
    wqb = _block_diag_weights(wq)
    wkb = _block_diag_weights(wk)
    wvb = _block_diag_weights(wv)
    idn = np.broadcast_to(np.eye(128, dtype=np.float32), (NCH, 128, 128))
    wall = np.ascontiguousarray(
        np.concatenate([wkb, wqb, wvb, idn], axis=2)
    ).astype(bf)
    relb = rel_emb.reshape(G, Cg, K * K).reshape(NCH, 128, K * K)
    qeb = q_emb.reshape(NCH, 128, 1)
    rqb = np.ascontiguousarray(np.concatenate([relb, qeb], axis=2))

    xp = np.pad(x, ((0, 0), (P, P), (P, P), (0, 0)))  # (B, 60, 60, C)

    in_maps = []
    for core in range(NCORES):
        b, half = divmod(core, 2)
        sh = xp[b, HS * half : HS * half + MR]         # (32, 60, C)
        xt = np.ascontiguousarray(sh.reshape(SP, C).T).reshape(NCH, 128, SP)
        in_maps.append(
            {
                "xt": xt.astype(bf),
                "wall": wall,
                "rqb": rqb,
            }
        )
    return in_maps


def kernel(**inputs):
    in_maps = _make_in_maps(inputs)

    if "nc" not in _NC_CACHE:
        _NC_CACHE["nc"] = build_nc()
    nc = _NC_CACHE["nc"]

    res = run_bass_kernel_spmd(nc, in_maps, core_ids=list(range(NCORES)))

    out = np.empty((B, H, W, C), np.float32)
    for core in range(NCORES):
        b, half = divmod(core, 2)
        den = res.results[core]["dd"].astype(np.float32).reshape(C, HS, W)
        num = res.results[core]["nd"].astype(np.float32).reshape(C, HS, W)
        o = num / den
        out[b, HS * half : HS * half + HS] = o.transpose(1, 2, 0)
    return out


# revision 17
# speedup vs baseline: 1.0989x; 1.0605x over previous
"""Trainium2 Bass kernel for AttentionConvFull (local 5x5 window attention
with per-channel softmax, grouped 1x1 conv projections).

Sharding: 8 cores = batch(4) x H-halves(2). Each core gets a 32-row halo'd,
zero-padded slice of x, pre-transposed on host to channel-major [256, 32*60].
No collectives needed.

Per-core dataflow (2 channel-chunks of 128 partitions each), per dj-column
group of 5 window taps:
  DVE : t_j = (k_j + rel_j) * q  -- one fused scalar_tensor_tensor per tap
        w5  = e5 * v5            -- one tensor_tensor over all 5 taps
        (window views of the v map via a custom overlapping strided AP)
  ACT : e5  = exp(t5)            -- one activation over all 5 taps
  PE  : den += e_j, num += e_j*v_j via per-tap identity matmuls into PSUM
        (one matmul is ISA-capped at 512 out elems; back-to-back matmuls
        pipeline at streaming rate, so per-tap instructions are fine).
  Epilogue: den/num cast PSUM->SBUF bf16 and DMA'd out; the division
  num/den and the layout transpose happen on host.

Measured engine balance (per core): DVE ~145us busy (saturated, the
critical engine), ACT ~93us, PE ~85us-equivalent. The fused STT runs at
1x DVE mode but does 2 ALU ops/elem — same throughput as two 2x passes;
splitting it into TS(4x)+TT(2x) passes measured slower end-to-end due to
SBUF contention inflating ACT's exp.
"""

import numpy as np
import ml_dtypes

import concourse.bass as bass
import concourse.tile as tile
from concourse import bacc, mybir
from concourse.ap import AP
from concourse.bass_utils import run_bass_kernel_spmd

F32 = mybir.dt.float32
BF16 = mybir.dt.bfloat16

K = 5
G = 8
B, H, W, C = 4, 56, 56, 256
Cg = C // G            # 32
P = K // 2             # 2
HS = H // 2            # 28 output rows per shard
MR = HS + 2 * P        # 32 map rows
MC = W + 2 * P         # 60 map cols
SP = MR * MC           # 1920 map spatial
OP = HS * W            # 1568 output spatial per shard
NCH = 2                # channel chunks of 128 partitions
NCORES = 8
HALF = OP // 2         # 784: PSUM accumulator tile size


def _dedup_ldweights(nc):
    """Remove redundant PE weight reloads: consecutive InstLdweights that
    load the same stationary operand with no sync info."""
    removed = 0
    for blk in nc.main_func.blocks:
        last_sig = None
        keep = []
        for inst in blk.instructions:
            if isinstance(inst, mybir.InstLdweights):
                sig = " ".join(a.concise() for a in inst.ins)
                si = inst.sync_info
                clean = si is None or (
                    len(si.on_wait) == 0 and len(si.on_update) == 0
                )
                if sig == last_sig and clean:
                    removed += 1
                    continue
                last_sig = sig
            elif isinstance(inst, mybir.InstMatmult):
                if len(inst.ins) > 1:
                    wsig = inst.ins[1].concise()
                    if wsig != last_sig:
                        last_sig = wsig
            keep.append(inst)
        blk.instructions[:] = keep
    return removed


def _strided_view(base, extra_offset, dims):
    """Custom strided/broadcast view of a 2D [128, N] tile AP.
    dims: list of (stride, num) free dims, outer->inner."""
    pairs = list(base.ap)
    pstride, pnum = pairs[0]
    return AP(
        base.tensor,
        base.offset + extra_offset,
        [[pstride, pnum]] + [[s, n] for s, n in dims],
    )


def build_nc():
    nc = bacc.Bacc(
        "TRN2", target_bir_lowering=False, debug=False, num_devices=NCORES
    )

    xt_d = nc.dram_tensor("xt", [NCH, 128, SP], BF16, kind="ExternalInput").ap()
    wq_d = nc.dram_tensor("wqb", [NCH, 128, 128], BF16, kind="ExternalInput").ap()
    wk_d = nc.dram_tensor("wkb", [NCH, 128, 128], BF16, kind="ExternalInput").ap()
    wv_d = nc.dram_tensor("wvb", [NCH, 128, 128], BF16, kind="ExternalInput").ap()
    rel_d = nc.dram_tensor("relb", [NCH, 128, K * K], F32, kind="ExternalInput").ap()
    qe_d = nc.dram_tensor("qeb", [NCH, 128, 1], F32, kind="ExternalInput").ap()
    id_d = nc.dram_tensor("idn", [128, 128], BF16, kind="ExternalInput").ap()
    den_d = nc.dram_tensor("dd", [NCH, 128, OP], BF16, kind="ExternalOutput").ap()
    num_d = nc.dram_tensor("nd", [NCH, 128, OP], BF16, kind="ExternalOutput").ap()

    with tile.TileContext(nc) as tc:
        with (
            tc.tile_pool(name="consts", bufs=1) as consts,
            tc.tile_pool(name="weights", bufs=2) as wpool,
            tc.tile_pool(name="xin", bufs=2) as xpool,
            tc.tile_pool(name="maps", bufs=2) as mpool,
            tc.tile_pool(name="tgrp", bufs=2) as tpool,
            tc.tile_pool(name="egrp", bufs=2) as epool,
            tc.tile_pool(name="wgrp", bufs=2) as wgpool,
            tc.tile_pool(name="epi", bufs=2) as opool,
            tc.tile_pool(name="pacc", bufs=4, space=bass.MemorySpace.PSUM) as apsum,
        ):
            ident = consts.tile([128, 128], BF16, tag="ident")
            nc.sync.dma_start(ident[:], id_d)

            kmaps, komaps, vmaps, vomaps, qflats, rels = (
                [None, None] for _ in range(6)
            )
            wts, qes, xsbs = [None, None], [None, None], [None, None]

            def emit_inputs(c):
                x_sb = xpool.tile([128, SP], BF16, tag="x", name=f"x{c}")
                nc.sync.dma_start(x_sb[:], xt_d[c])
                xsbs[c] = x_sb
                wd = {}
                for nm, d in (("wk", wk_d), ("wq", wq_d), ("wv", wv_d)):
                    t = wpool.tile([128, 128], BF16, tag=nm, name=f"{nm}{c}")
                    nc.sync.dma_start(t[:], d[c])
                    wd[nm] = t
                wts[c] = wd
                rel_sb = wpool.tile([128, K * K], F32, tag="rel", name=f"rel{c}")
                nc.sync.dma_start(rel_sb[:], rel_d[c])
                rels[c] = rel_sb
                qe_sb = wpool.tile([128, 1], F32, tag="qe", name=f"qe{c}")
                nc.sync.dma_start(qe_sb[:], qe_d[c])
                qes[c] = qe_sb

            def emit_proj(c):
                x_sb = xsbs[c]
                k_bf = mpool.tile([128, SP], BF16, tag="k", name=f"k{c}")
                v_bf = mpool.tile([128, SP], BF16, tag="v", name=f"v{c}")
                qf = mpool.tile([128, OP], BF16, tag="qf", name=f"qf{c}")
                NS = 2
                SL = SP // NS  # 960 (16 map rows per slice)
                # k first (unblocks the DVE j-loop), then q, then v; casts
                # split DVE/ACT so the two slices of each map cast in
                # parallel and the startup is not ACT-serial.
                for nm in ("wk", "wq", "wv"):
                    for s in range(NS):
                        lo = s * SL
                        rhs = x_sb[:, lo : lo + SL]
                        ps = apsum.tile(
                            [128, SL], F32, tag="acc", name=f"pp{c}{s}{nm}"
                        )
                        for mlo, mn in ((0, 512), (512, SL - 512)):
                            nc.tensor.matmul(
                                ps[:, mlo : mlo + mn],
                                wts[c][nm][:],
                                rhs[:, mlo : mlo + mn],
                                start=True,
                                stop=True,
                            )
                        if nm == "wq":
                            # interior rows/cols of this 16-row band into
                            # flat q, fusing the q_emb per-partition bias
                            r0 = max(P, 16 * s)
                            r1 = min(MR - P, 16 * (s + 1))
                            src = ps[:].rearrange("p (h w) -> p h w", h=16)[
                                :, r0 - 16 * s : r1 - 16 * s, P : P + W
                            ]
                            dst = qf[:].rearrange("p (h w) -> p h w", h=HS)[
                                :, r0 - P : r1 - P, :
                            ]
                            nc.scalar.activation(
                                dst,
                                src,
                                mybir.ActivationFunctionType.Identity,
                                bias=qes[c][:],
                            )
                        else:
                            # chunk 0: k casts (and v-s0) on DVE so the
                            # j-loop STT chain is gated only by k+q, not by
                            # ACT's serial cast queue. chunk 1: all on ACT
                            # (DVE is busy with chunk-0's j-loop by then).
                            dst_map = k_bf if nm == "wk" else v_bf
                            on_dve = c == 0 and (nm == "wk" or s == 0)
                            if on_dve:
                                nc.vector.tensor_copy(
                                    dst_map[:, lo : lo + SL], ps[:]
                                )
                            else:
                                nc.scalar.copy(dst_map[:, lo : lo + SL], ps[:])

                # 1-elem-shifted copies so odd window columns keep 4B align
                k_od = mpool.tile([128, SP], BF16, tag="ko", name=f"ko{c}")
                v_od = mpool.tile([128, SP], BF16, tag="vo", name=f"vo{c}")
                nc.sync.dma_start(k_od[:, : SP - 1], k_bf[:, 1:])
                nc.sync.dma_start(v_od[:, : SP - 1], v_bf[:, 1:])
                kmaps[c], komaps[c] = k_bf, k_od
                vmaps[c], vomaps[c] = v_bf, v_od
                qflats[c] = qf

            def emit_group(c, dj, den, num, di0=0, ndi=K):
                dje = dj - (dj % 2)
                kc = kmaps[c] if dj % 2 == 0 else komaps[c]
                vc = vmaps[c] if dj % 2 == 0 else vomaps[c]
                nm = f"{c}{dj}{di0}"
                tg = tpool.tile([128, ndi * OP], BF16, tag="t", name=f"t{nm}")
                eg = epool.tile([128, ndi * OP], BF16, tag="e", name=f"e{nm}")
                wg = wgpool.tile([128, ndi * OP], BF16, tag="w", name=f"w{nm}")

                k3 = kc[:].rearrange("p (h w) -> p h w", h=MR)
                qf3 = qflats[c][:].rearrange("p (h w) -> p h w", h=HS)
                tg4 = tg[:].rearrange("p (j h w) -> p j h w", j=ndi, h=HS)
                # t_j = (k_j + rel_j) * q, one fused DVE op per tap. The STT
                # runs at 1x DVE mode, but it performs 2 ALU ops per element
                # there — same throughput as two 2x-mode passes, with fewer
                # instructions and less SBUF pressure. (Moving one tap's
                # rel-add to ACT + a 2x TT product measured no better.)
                for idx in range(ndi):
                    di = di0 + idx
                    j = di * K + dj
                    kv = k3[:, di : di + HS, dje : dje + W]
                    nc.vector.scalar_tensor_tensor(
                        tg4[:, idx],
                        kv,
                        rels[c][:, j : j + 1],
                        qf3,
                        mybir.AluOpType.add,
                        mybir.AluOpType.mult,
                    )

                nc.scalar.activation(
                    eg[:], tg[:], mybir.ActivationFunctionType.Exp
                )

                eg4 = eg[:].rearrange("p (j h w) -> p j h w", j=ndi, h=HS)
                wg4 = wg[:].rearrange("p (j h w) -> p j h w", j=ndi, h=HS)
                vwin = _strided_view(
                    vc[:], dje + di0 * MC, [(MC, ndi), (MC, HS), (1, W)]
                )
                nc.vector.tensor_tensor(
                    wg4, eg4, vwin, mybir.AluOpType.mult
                )

                eg3 = eg[:].rearrange("p (j n) -> p j n", j=ndi)
                wg3 = wg[:].rearrange("p (j n) -> p j n", j=ndi)
                # per-tap matmuls (ISA caps one matmul at 512 out elems);
                # alternate PSUM regions so consecutive mms hit different
                # banks: h0-512, h1-512, h0-272, h1-272 per tap.
                for acc, src3 in ((den, eg3), (num, wg3)):
                    for idx in range(ndi):
                        di = di0 + idx
                        for lo, n in ((0, 512), (512, HALF - 512)):
                            for h in range(2):
                                base = h * HALF
                                nc.tensor.matmul(
                                    acc[h][:, lo : lo + n],
                                    ident[:],
                                    src3[:, idx, base + lo : base + lo + n],
                                    start=dj == 0 and di == 0,
                                    stop=dj == K - 1 and di == K - 1,
                                )

            def emit_epilogue(c, den, num):
                den_sb = opool.tile([128, OP], BF16, tag="osb", name=f"dsb{c}")
                num_sb = opool.tile([128, OP], BF16, tag="osb", name=f"nsb{c}")
                # chunk 0's epilogue runs mid-kernel: keep it off the
                # saturated DVE. chunk 1's runs in the tail where DVE idles.
                for h in range(2):
                    base = h * HALF
                    if c == 1:
                        nc.vector.tensor_copy(
                            den_sb[:, base : base + HALF], den[h][:]
                        )
                    else:
                        nc.scalar.copy(
                            den_sb[:, base : base + HALF], den[h][:]
                        )
                    nc.scalar.copy(num_sb[:, base : base + HALF], num[h][:])
                nc.sync.dma_start(den_d[c], den_sb[:])
                nc.sync.dma_start(num_d[c], num_sb[:])

            # ---- emission schedule ----
            # both projections first (PSUM pool is shared with den/num, so
            # chunk-1 proj scratch must rotate through before den/num-c0
            # pin all four buffers). DVE still starts chunk-0 STTs as soon
            # as k/q maps land — nothing else occupies it.
            emit_inputs(0)
            emit_proj(0)
            emit_inputs(1)
            emit_proj(1)
            for c in range(NCH):
                den = [
                    apsum.tile([128, HALF], F32, tag="acc", name=f"den{c}{h}")
                    for h in range(2)
                ]
                num = [
                    apsum.tile([128, HALF], F32, tag="acc", name=f"num{c}{h}")
                    for h in range(2)
                ]
                for dj in range(K):
                    if c == 1 and dj == K - 1:
                        # split the final group so the tail drain
                        # (exp -> w-mult -> matmuls -> epilogue) runs on
                        # smaller quanta
                        emit_group(c, dj, den, num, 0, 3)
                        emit_group(c, dj, den, num, 3, 2)
                    else:
                        emit_group(c, dj, den, num)
                emit_epilogue(c, den, num)

    nc.compile()
    _dedup_ldweights(nc)
    return nc


def _block_diag_weights(w):
    """w: (G, Cg_out, Cg_in) -> lhsT layout [NCH, 128, 128] where
    lhsT[c, ci, co] = w[g, co%32, ci%32] for matching 32-blocks."""
    out = np.zeros((NCH, 128, 128), np.float32)
    for c in range(NCH):
        for g4 in range(4):
            g = c * 4 + g4
            blk = w[g]  # (Cg_out, Cg_in)
            out[c, g4 * 32 : (g4 + 1) * 32, g4 * 32 : (g4 + 1) * 32] = blk.T
    return out


_NC_CACHE = {}


def _make_in_maps(inputs):
    x = np.asarray(inputs["x"], np.float32)
    wq = np.asarray(inputs["wq"], np.float32)
    wk = np.asarray(inputs["wk"], np.float32)
    wv = np.asarray(inputs["wv"], np.float32)
    rel_emb = np.asarray(inputs["rel_emb"], np.float32)
    q_emb = np.asarray(inputs["q_emb"], np.float32)

    bf = ml_dtypes.bfloat16
    wqb = _block_diag_weights(wq).astype(bf)
    wkb = _block_diag_weights(wk).astype(bf)
    wvb = _block_diag_weights(wv).astype(bf)
    relb = np.ascontiguousarray(
        rel_emb.reshape(G, Cg, K * K).reshape(NCH, 128, K * K)
    )
    qeb = np.ascontiguousarray(q_emb.reshape(NCH, 128, 1))
    idn = np.eye(128, dtype=bf)

    xp = np.pad(x, ((0, 0), (P, P), (P, P), (0, 0)))  # (B, 60, 60, C)

    in_maps = []
    for core in range(NCORES):
        b, half = divmod(core, 2)
        sh = xp[b, HS * half : HS * half + MR]         # (32, 60, C)
        xt = np.ascontiguousarray(sh.reshape(SP, C).T).reshape(NCH, 128, SP)
        in_maps.append(
            {
                "xt": xt.astype(bf),
                "wqb": wqb,
                "wkb": wkb,
                "wvb": wvb,
                "relb": relb,
                "qeb": qeb,
                "idn": idn,
            }
        )
    return in_maps


def kernel(**inputs):
    in_maps = _make_in_maps(inputs)

    if "nc" not in _NC_CACHE:
        _NC_CACHE["nc"] = build_nc()
    nc = _NC_CACHE["nc"]

    res = run_bass_kernel_spmd(nc, in_maps, core_ids=list(range(NCORES)))

    out = np.empty((B, H, W, C), np.float32)
    for core in range(NCORES):
        b, half = divmod(core, 2)
        den = res.results[core]["dd"].astype(np.float32).reshape(C, HS, W)
        num = res.results[core]["nd"].astype(np.float32).reshape(C, HS, W)
        o = num / den
        out[b, HS * half : HS * half + HS] = o.transpose(1, 2, 0)
    return out



# revision 18
# speedup vs baseline: 1.2275x; 1.1170x over previous
"""Trainium2 Bass kernel for AttentionConvFull (local 5x5 window attention
with per-channel softmax, grouped 1x1 conv projections).

Sharding: 8 cores = batch(4) x H-halves(2). Each core gets a 32-row halo'd,
zero-padded slice of x, pre-transposed on host to channel-major [256, 32*60].
No collectives needed.

v4 dataflow (vs the STT baseline): the fused scalar_tensor_tensor ran at
1x DVE mode (1.04 ns/elem measured). Split into:
  kr_j = k_j + rel_j   -- tensor_scalar per tap: 4x DVE mode (0.26 ns/elem)
                          or ACT identity+bias (split to balance engines)
  t    = kr (.) q      -- one grouped 2x tensor_tensor per dj-group,
                          q broadcast over taps via stride-0 AP dim
  e    = exp(t)        -- ACT, grouped
  w    = e (.) v_win   -- 2x tensor_tensor (overlapping strided v view)
  den += e_j, num += w_j on PE via identity matmuls (measured: overlapped
  back-to-back matmuls stream at 0.417 ns/col when the p-state stays hot).
Epilogue: den/num cast PSUM->SBUF bf16, DMA out; final num/den on host.
"""

import numpy as np
import ml_dtypes

import concourse.bass as bass
import concourse.tile as tile
from concourse import bacc, mybir
from concourse.ap import AP
from concourse.bass_utils import run_bass_kernel_spmd

F32 = mybir.dt.float32
BF16 = mybir.dt.bfloat16

K = 5
G = 8
B, H, W, C = 4, 56, 56, 256
Cg = C // G            # 32
P = K // 2             # 2
HS = H // 2            # 28 output rows per shard
MR = HS + 2 * P        # 32 map rows
MC = W + 2 * P         # 60 map cols
SP = MR * MC           # 1920 map spatial
OP = HS * W            # 1568 output spatial per shard
NCH = 2                # channel chunks of 128 partitions
NCORES = 8
HALF = OP // 2         # 784: PSUM accumulator tile size


def _dedup_ldweights(nc):
    """Remove redundant PE weight reloads: consecutive InstLdweights that
    load the same stationary operand with no sync info."""
    removed = 0
    for blk in nc.main_func.blocks:
        last_sig = None
        keep = []
        for inst in blk.instructions:
            if isinstance(inst, mybir.InstLdweights):
                sig = " ".join(a.concise() for a in inst.ins)
                si = inst.sync_info
                clean = si is None or (
                    len(si.on_wait) == 0 and len(si.on_update) == 0
                )
                if sig == last_sig and clean:
                    removed += 1
                    continue
                last_sig = sig
            elif isinstance(inst, mybir.InstMatmult):
                if len(inst.ins) > 1:
                    wsig = inst.ins[1].concise()
                    if wsig != last_sig:
                        last_sig = wsig
            keep.append(inst)
        blk.instructions[:] = keep
    return removed


def _strided_view(base, extra_offset, dims):
    """Custom strided/broadcast view of a 2D [128, N] tile AP.
    dims: list of (stride, num) free dims, outer->inner."""
    pairs = list(base.ap)
    pstride, pnum = pairs[0]
    return AP(
        base.tensor,
        base.offset + extra_offset,
        [[pstride, pnum]] + [[s, n] for s, n in dims],
    )


def build_nc():
    nc = bacc.Bacc(
        "TRN2", target_bir_lowering=False, debug=False, num_devices=NCORES
    )

    xt_d = nc.dram_tensor("xt", [NCH, 128, SP], BF16, kind="ExternalInput").ap()
    # weights batched into single DMAs: [wk | wq | wv | ident] bf16 and
    # [rel | qe] f32 (DMA issue on the sync queue costs ~600ns each)
    wall_d = nc.dram_tensor(
        "wall", [NCH, 128, 128 * 4], BF16, kind="ExternalInput"
    ).ap()
    rq_d = nc.dram_tensor(
        "rqb", [NCH, 128, K * K + 1], F32, kind="ExternalInput"
    ).ap()
    den_d = nc.dram_tensor("dd", [NCH, 128, OP], BF16, kind="ExternalOutput").ap()
    num_d = nc.dram_tensor("nd", [NCH, 128, OP], BF16, kind="ExternalOutput").ap()

    with tile.TileContext(nc) as tc:
        with (
            tc.tile_pool(name="weights", bufs=2) as wpool,
            tc.tile_pool(name="xin", bufs=2) as xpool,
            tc.tile_pool(name="maps", bufs=2) as mpool,
            tc.tile_pool(name="krgrp", bufs=2) as krpool,
            tc.tile_pool(name="tgrp", bufs=2) as tpool,
            tc.tile_pool(name="egrp", bufs=2) as epool,
            tc.tile_pool(name="wgrp", bufs=3) as wgpool,
            tc.tile_pool(name="epi", bufs=2) as opool,
            tc.tile_pool(name="pacc", bufs=4, space=bass.MemorySpace.PSUM) as apsum,
        ):
            kmaps, komaps, vmaps, vomaps, qflats, rels = (
                [None, None] for _ in range(6)
            )
            wts, qes, xsbs = [None, None], [None, None], [None, None]
            idents = [None, None]

            def emit_inputs(c):
                x_sb = xpool.tile([128, SP], BF16, tag="x", name=f"x{c}")
                nc.sync.dma_start(x_sb[:], xt_d[c])
                xsbs[c] = x_sb
                wall = wpool.tile([128, 128 * 4], BF16, tag="wall", name=f"wall{c}")
                nc.sync.dma_start(wall[:], wall_d[c])
                wts[c] = {
                    "wk": wall[:, 0:128],
                    "wq": wall[:, 128:256],
                    "wv": wall[:, 256:384],
                }
                idents[c] = wall[:, 384:512]
                rq_sb = wpool.tile([128, K * K + 1], F32, tag="rq", name=f"rq{c}")
                nc.sync.dma_start(rq_sb[:], rq_d[c])
                rels[c] = rq_sb
                qes[c] = rq_sb[:, K * K : K * K + 1]

            def emit_proj(c):
                x_sb = xsbs[c]
                k_bf = mpool.tile([128, SP], BF16, tag="k", name=f"k{c}")
                v_bf = mpool.tile([128, SP], BF16, tag="v", name=f"v{c}")
                qf = mpool.tile([128, OP], BF16, tag="qf", name=f"qf{c}")
                NS = 2
                SL = SP // NS  # 960 (16 map rows per slice)
                # k first (unblocks the DVE j-loop), then q, then v.
                for nm in ("wk", "wq", "wv"):
                    for s in range(NS):
                        lo = s * SL
                        rhs = x_sb[:, lo : lo + SL]
                        ps = apsum.tile(
                            [128, SL], F32, tag="acc", name=f"pp{c}{s}{nm}"
                        )
                        for mlo, mn in ((0, 512), (512, SL - 512)):
                            nc.tensor.matmul(
                                ps[:, mlo : mlo + mn],
                                wts[c][nm],
                                rhs[:, mlo : mlo + mn],
                                start=True,
                                stop=True,
                            )
                        if nm == "wq":
                            # interior rows/cols of this 16-row band into
                            # flat q, fusing the q_emb per-partition bias
                            r0 = max(P, 16 * s)
                            r1 = min(MR - P, 16 * (s + 1))
                            src = ps[:].rearrange("p (h w) -> p h w", h=16)[
                                :, r0 - 16 * s : r1 - 16 * s, P : P + W
                            ]
                            dst = qf[:].rearrange("p (h w) -> p h w", h=HS)[
                                :, r0 - P : r1 - P, :
                            ]
                            nc.scalar.activation(
                                dst,
                                src,
                                mybir.ActivationFunctionType.Identity,
                                bias=qes[c],
                            )
                        else:
                            # k casts split across ACT/DVE so the map is
                            # ready fastest (it gates the whole j-loop);
                            # v casts on ACT (needed later).
                            dst_map = k_bf if nm == "wk" else v_bf
                            if nm == "wk" and s == 1:
                                nc.vector.tensor_copy(
                                    dst_map[:, lo : lo + SL], ps[:]
                                )
                            else:
                                nc.scalar.copy(dst_map[:, lo : lo + SL], ps[:])

                # 1-elem-shifted copies so odd window columns keep 4B align
                k_od = mpool.tile([128, SP], BF16, tag="ko", name=f"ko{c}")
                v_od = mpool.tile([128, SP], BF16, tag="vo", name=f"vo{c}")
                nc.sync.dma_start(k_od[:, : SP - 1], k_bf[:, 1:])
                nc.sync.dma_start(v_od[:, : SP - 1], v_bf[:, 1:])
                kmaps[c], komaps[c] = k_bf, k_od
                vmaps[c], vomaps[c] = v_bf, v_od
                qflats[c] = qf

            def emit_group(c, dj, den, num, di0=0, ndi=K, nact=0):
                dje = dj - (dj % 2)
                kc = kmaps[c] if dj % 2 == 0 else komaps[c]
                vc = vmaps[c] if dj % 2 == 0 else vomaps[c]
                nm = f"{c}{dj}{di0}"
                kr = krpool.tile([128, ndi * OP], BF16, tag="kr", name=f"kr{nm}")
                tg = tpool.tile([128, ndi * OP], BF16, tag="t", name=f"t{nm}")
                eg = epool.tile([128, ndi * OP], BF16, tag="e", name=f"e{nm}")
                wg = wgpool.tile([128, ndi * OP], BF16, tag="w", name=f"w{nm}")

                k3 = kc[:].rearrange("p (h w) -> p h w", h=MR)
                kr4 = kr[:].rearrange("p (j h w) -> p j h w", j=ndi, h=HS)
                # kr_j = k_j + rel_j: per-tap add of a per-partition scalar.
                # DVE tensor_scalar runs at 4x; ACT identity+bias costs ~1
                # elem/cycle. Split taps to balance engine load.
                n_act_rel = nact
                for idx in range(ndi):
                    di = di0 + idx
                    j = di * K + dj
                    kv = k3[:, di : di + HS, dje : dje + W]
                    if idx < n_act_rel:
                        nc.scalar.activation(
                            kr4[:, idx],
                            kv,
                            mybir.ActivationFunctionType.Identity,
                            bias=rels[c][:, j : j + 1],
                        )
                    else:
                        nc.vector.tensor_scalar(
                            kr4[:, idx],
                            kv,
                            rels[c][:, j : j + 1],
                            None,
                            mybir.AluOpType.add,
                        )

                # t = kr (.) q grouped 2x tensor_tensor; q broadcast over the
                # tap dim via a stride-0 AP dim. First group: per-tap, so the
                # chain to the first PE matmul is short (pipeline fill).
                tg3 = tg[:].rearrange("p (j n) -> p j n", j=ndi)
                kr3 = kr[:].rearrange("p (j n) -> p j n", j=ndi)
                first = c == 0 and dj == 0
                eg4 = eg[:].rearrange("p (j h w) -> p j h w", j=ndi, h=HS)
                wg4 = wg[:].rearrange("p (j h w) -> p j h w", j=ndi, h=HS)
                if first:
                    for sl in ((0, 1), (1, 3), (3, 5)):
                        a, b = sl
                        qb = _strided_view(
                            qflats[c][:], 0, [(0, b - a), (1, OP)]
                        )
                        nc.vector.tensor_tensor(
                            tg3[:, a:b], kr3[:, a:b], qb, mybir.AluOpType.mult
                        )
                        nc.scalar.activation(
                            eg[:, a * OP : b * OP],
                            tg[:, a * OP : b * OP],
                            mybir.ActivationFunctionType.Exp,
                        )
                        vwin = _strided_view(
                            vc[:],
                            dje + (di0 + a) * MC,
                            [(MC, b - a), (MC, HS), (1, W)],
                        )
                        nc.vector.tensor_tensor(
                            wg4[:, a:b], eg4[:, a:b], vwin, mybir.AluOpType.mult
                        )
                else:
                    qb = _strided_view(qflats[c][:], 0, [(0, ndi), (1, OP)])
                    nc.vector.tensor_tensor(
                        tg3, kr3, qb, mybir.AluOpType.mult
                    )
                    nc.scalar.activation(
                        eg[:], tg[:], mybir.ActivationFunctionType.Exp
                    )
                    vwin = _strided_view(
                        vc[:], dje + di0 * MC, [(MC, ndi), (MC, HS), (1, W)]
                    )
                    nc.vector.tensor_tensor(
                        wg4, eg4, vwin, mybir.AluOpType.mult
                    )

                eg3 = eg[:].rearrange("p (j n) -> p j n", j=ndi)
                wg3 = wg[:].rearrange("p (j n) -> p j n", j=ndi)
                # per-tap matmuls (ISA caps one matmul at 512 out elems);
                # alternate PSUM regions so consecutive mms hit different
                # banks: h0-512, h1-512, h0-272, h1-272 per tap.
                for acc, src3 in ((den, eg3), (num, wg3)):
                    for idx in range(ndi):
                        di = di0 + idx
                        for lo, n in ((0, 512), (512, HALF - 512)):
                            for h in range(2):
                                base = h * HALF
                                nc.tensor.matmul(
                                    acc[h][:, lo : lo + n],
                                    idents[c],
                                    src3[:, idx, base + lo : base + lo + n],
                                    start=dj == 0 and di == 0,
                                    stop=dj == K - 1 and di == K - 1,
                                )

            def emit_epilogue(c, den, num):
                den_sb = opool.tile([128, OP], BF16, tag="osb", name=f"dsb{c}")
                num_sb = opool.tile([128, OP], BF16, tag="osb", name=f"nsb{c}")
                # split the PSUM->SBUF drain across both engines so the
                # PSUM banks free up fast (chunk 1's accumulators wait on
                # chunk 0's drain).
                for h in range(2):
                    base = h * HALF
                    if h == 0:
                        nc.vector.tensor_copy(
                            den_sb[:, base : base + HALF], den[h][:]
                        )
                        nc.scalar.copy(num_sb[:, base : base + HALF], num[h][:])
                    else:
                        nc.scalar.copy(
                            den_sb[:, base : base + HALF], den[h][:]
                        )
                        nc.vector.tensor_copy(
                            num_sb[:, base : base + HALF], num[h][:]
                        )
                nc.sync.dma_start(den_d[c], den_sb[:])
                nc.sync.dma_start(num_d[c], num_sb[:])

            # ---- emission schedule ----
            emit_inputs(0)
            emit_proj(0)
            emit_inputs(1)
            emit_proj(1)
            accs = []
            for c in range(NCH):
                den = [
                    apsum.tile([128, HALF], F32, tag="acc", name=f"den{c}{h}")
                    for h in range(2)
                ]
                num = [
                    apsum.tile([128, HALF], F32, tag="acc", name=f"num{c}{h}")
                    for h in range(2)
                ]
                accs.append((den, num))
                for dj in range(K):
                    nact = 2 - (dj % 2)
                    if c == 1 and dj == K - 1:
                        # split the final group so the tail drain runs on
                        # smaller quanta
                        emit_group(c, dj, den, num, 0, 3)
                        emit_group(c, dj, den, num, 3, 1)
                        emit_group(c, dj, den, num, 4, 1)
                    else:
                        emit_group(c, dj, den, num, 0, K, nact)
                emit_epilogue(c, den, num)

    nc.compile()
    _dedup_ldweights(nc)
    return nc


def _block_diag_weights(w):
    """w: (G, Cg_out, Cg_in) -> lhsT layout [NCH, 128, 128] where
    lhsT[c, ci, co] = w[g, co%32, ci%32] for matching 32-blocks."""
    out = np.zeros((NCH, 128, 128), np.float32)
    for c in range(NCH):
        for g4 in range(4):
            g = c * 4 + g4
            blk = w[g]  # (Cg_out, Cg_in)
            out[c, g4 * 32 : (g4 + 1) * 32, g4 * 32 : (g4 + 1) * 32] = blk.T
    return out


_NC_CACHE = {}


def _make_in_maps(inputs):
    x = np.asarray(inputs["x"], np.float32)
    wq = np.asarray(inputs["wq"], np.float32)
    wk = np.asarray(inputs["wk"], np.float32)
    wv = np.asarray(inputs["wv"], np.float32)
    rel_emb = np.asarray(inputs["rel_emb"], np.float32)
    q_emb = np.asarray(inputs["q_emb"], np.float32)

    bf = ml_dtypes.bfloat16
    wqb = _block_diag_weights(wq)
    wkb = _block_diag_weights(wk)
    wvb = _block_diag_weights(wv)
    idn = np.broadcast_to(np.eye(128, dtype=np.float32), (NCH, 128, 128))
    wall = np.ascontiguousarray(
        np.concatenate([wkb, wqb, wvb, idn], axis=2)
    ).astype(bf)
    relb = rel_emb.reshape(G, Cg, K * K).reshape(NCH, 128, K * K)
    qeb = q_emb.reshape(NCH, 128, 1)
    rqb = np.ascontiguousarray(np.concatenate([relb, qeb], axis=2))

    xp = np.pad(x, ((0, 0), (P, P), (P, P), (0, 0)))  # (B, 60, 60, C)

    in_maps = []
    for core in range(NCORES):
        b, half = divmod(core, 2)
        sh = xp[b, HS * half : HS * half + MR]         # (32, 60, C)
        xt = np.ascontiguousarray(sh.reshape(SP, C).T).reshape(NCH, 128, SP)
        in_maps.append(
            {
                "xt": xt.astype(bf),
                "wall": wall,
                "rqb": rqb,
            }
        )
    return in_maps


def kernel(**inputs):
    in_maps = _make_in_maps(inputs)

    if "nc" not in _NC_CACHE:
        _NC_CACHE["nc"] = build_nc()
    nc = _NC_CACHE["nc"]

    res = run_bass_kernel_spmd(nc, in_maps, core_ids=list(range(NCORES)))

    out = np.empty((B, H, W, C), np.float32)
    for core in range(NCORES):
        b, half = divmod(core, 2)
        den = res.results[core]["dd"].astype(np.float32).reshape(C, HS, W)
        num = res.results[core]["nd"].astype(np.float32).reshape(C, HS, W)
        o = num / den
        out[b, HS * half : HS * half + HS] = o.transpose(1, 2, 0)
    return out


# revision 20
# speedup vs baseline: 1.2344x; 1.0056x over previous
"""Trainium2 Bass kernel for AttentionConvFull (local 5x5 window attention
with per-channel softmax, grouped 1x1 conv projections).

Sharding: 8 cores = batch(4) x H-halves(2). Each core gets a 32-row halo'd,
zero-padded slice of x, pre-transposed on host to channel-major [256, 32*60].
No collectives needed.

v4 dataflow (vs the STT baseline): the fused scalar_tensor_tensor ran at
1x DVE mode (1.04 ns/elem measured). Split into:
  kr_j = k_j + rel_j   -- tensor_scalar per tap: 4x DVE mode (0.26 ns/elem)
                          or ACT identity+bias (split to balance engines)
  t    = kr (.) q      -- one grouped 2x tensor_tensor per dj-group,
                          q broadcast over taps via stride-0 AP dim
  e    = exp(t)        -- ACT, grouped
  w    = e (.) v_win   -- 2x tensor_tensor (overlapping strided v view)
  den += e_j, num += w_j on PE via identity matmuls (measured: overlapped
  back-to-back matmuls stream at 0.417 ns/col when the p-state stays hot).
Epilogue: den/num cast PSUM->SBUF bf16, DMA out; final num/den on host.
"""

import numpy as np
import ml_dtypes

import concourse.bass as bass
import concourse.tile as tile
from concourse import bacc, mybir
from concourse.ap import AP
from concourse.bass_utils import run_bass_kernel_spmd

F32 = mybir.dt.float32
BF16 = mybir.dt.bfloat16

K = 5
G = 8
B, H, W, C = 4, 56, 56, 256
Cg = C // G            # 32
P = K // 2             # 2
HS = H // 2            # 28 output rows per shard
MR = HS + 2 * P        # 32 map rows
MC = W + 2 * P         # 60 map cols
SP = MR * MC           # 1920 map spatial
OP = HS * W            # 1568 output spatial per shard
NCH = 2                # channel chunks of 128 partitions
NCORES = 8
HALF = OP // 2         # 784: PSUM accumulator tile size


def _dedup_ldweights(nc):
    """Remove redundant PE weight reloads: consecutive InstLdweights that
    load the same stationary operand with no sync info."""
    removed = 0
    for blk in nc.main_func.blocks:
        last_sig = None
        keep = []
        for inst in blk.instructions:
            if isinstance(inst, mybir.InstLdweights):
                sig = " ".join(a.concise() for a in inst.ins)
                si = inst.sync_info
                clean = si is None or (
                    len(si.on_wait) == 0 and len(si.on_update) == 0
                )
                if sig == last_sig and clean:
                    removed += 1
                    continue
                last_sig = sig
            elif isinstance(inst, mybir.InstMatmult):
                if len(inst.ins) > 1:
                    wsig = inst.ins[1].concise()
                    if wsig != last_sig:
                        last_sig = wsig
            keep.append(inst)
        blk.instructions[:] = keep
    return removed


def _strided_view(base, extra_offset, dims):
    """Custom strided/broadcast view of a 2D [128, N] tile AP.
    dims: list of (stride, num) free dims, outer->inner."""
    pairs = list(base.ap)
    pstride, pnum = pairs[0]
    return AP(
        base.tensor,
        base.offset + extra_offset,
        [[pstride, pnum]] + [[s, n] for s, n in dims],
    )


def build_nc():
    nc = bacc.Bacc(
        "TRN2", target_bir_lowering=False, debug=False, num_devices=NCORES
    )

    xt_d = nc.dram_tensor("xt", [NCH, 128, SP], BF16, kind="ExternalInput").ap()
    # weights batched into single DMAs: [wk | wq | wv | ident] bf16 and
    # [rel | qe] f32 (DMA issue on the sync queue costs ~600ns each)
    wall_d = nc.dram_tensor(
        "wall", [NCH, 128, 128 * 4], BF16, kind="ExternalInput"
    ).ap()
    rq_d = nc.dram_tensor(
        "rqb", [NCH, 128, K * K + 1], F32, kind="ExternalInput"
    ).ap()
    den_d = nc.dram_tensor("dd", [NCH, 128, OP], BF16, kind="ExternalOutput").ap()
    num_d = nc.dram_tensor("nd", [NCH, 128, OP], BF16, kind="ExternalOutput").ap()

    with tile.TileContext(nc) as tc:
        with (
            tc.tile_pool(name="weights", bufs=2) as wpool,
            tc.tile_pool(name="xin", bufs=2) as xpool,
            tc.tile_pool(name="maps", bufs=2) as mpool,
            tc.tile_pool(name="krgrp", bufs=2) as krpool,
            tc.tile_pool(name="tgrp", bufs=2) as tpool,
            tc.tile_pool(name="egrp", bufs=2) as epool,
            tc.tile_pool(name="wgrp", bufs=3) as wgpool,
            tc.tile_pool(name="epi", bufs=2) as opool,
            tc.tile_pool(name="pacc", bufs=4, space=bass.MemorySpace.PSUM) as apsum,
        ):
            kmaps, komaps, vmaps, vomaps, qflats, rels = (
                [None, None] for _ in range(6)
            )
            wts, qes, xsbs = [None, None], [None, None], [None, None]
            idents = [None, None]

            def emit_inputs(c):
                x_sb = xpool.tile([128, SP], BF16, tag="x", name=f"x{c}")
                nc.sync.dma_start(x_sb[:], xt_d[c])
                xsbs[c] = x_sb
                wall = wpool.tile([128, 128 * 4], BF16, tag="wall", name=f"wall{c}")
                nc.sync.dma_start(wall[:], wall_d[c])
                wts[c] = {
                    "wk": wall[:, 0:128],
                    "wq": wall[:, 128:256],
                    "wv": wall[:, 256:384],
                }
                idents[c] = wall[:, 384:512]
                rq_sb = wpool.tile([128, K * K + 1], F32, tag="rq", name=f"rq{c}")
                nc.sync.dma_start(rq_sb[:], rq_d[c])
                rels[c] = rq_sb
                qes[c] = rq_sb[:, K * K : K * K + 1]

            def emit_proj(c):
                x_sb = xsbs[c]
                k_bf = mpool.tile([128, SP], BF16, tag="k", name=f"k{c}")
                v_bf = mpool.tile([128, SP], BF16, tag="v", name=f"v{c}")
                qf = mpool.tile([128, OP], BF16, tag="qf", name=f"qf{c}")
                NS = 2
                SL = SP // NS  # 960 (16 map rows per slice)
                # k first (unblocks the DVE j-loop), then q, then v.
                for nm in ("wk", "wq", "wv"):
                    for s in range(NS):
                        lo = s * SL
                        rhs = x_sb[:, lo : lo + SL]
                        ps = apsum.tile(
                            [128, SL], F32, tag="acc", name=f"pp{c}{s}{nm}"
                        )
                        for mlo, mn in ((0, 512), (512, SL - 512)):
                            nc.tensor.matmul(
                                ps[:, mlo : mlo + mn],
                                wts[c][nm],
                                rhs[:, mlo : mlo + mn],
                                start=True,
                                stop=True,
                            )
                        if nm == "wq":
                            # interior rows/cols of this 16-row band into
                            # flat q, fusing the q_emb per-partition bias
                            r0 = max(P, 16 * s)
                            r1 = min(MR - P, 16 * (s + 1))
                            src = ps[:].rearrange("p (h w) -> p h w", h=16)[
                                :, r0 - 16 * s : r1 - 16 * s, P : P + W
                            ]
                            dst = qf[:].rearrange("p (h w) -> p h w", h=HS)[
                                :, r0 - P : r1 - P, :
                            ]
                            nc.scalar.activation(
                                dst,
                                src,
                                mybir.ActivationFunctionType.Identity,
                                bias=qes[c],
                            )
                        else:
                            # k casts split across ACT/DVE so the map is
                            # ready fastest (it gates the whole j-loop);
                            # v casts on ACT (needed later).
                            dst_map = k_bf if nm == "wk" else v_bf
                            if nm == "wk" and s == 1:
                                nc.vector.tensor_copy(
                                    dst_map[:, lo : lo + SL], ps[:]
                                )
                            else:
                                nc.scalar.copy(dst_map[:, lo : lo + SL], ps[:])

                # 1-elem-shifted copies so odd window columns keep 4B align
                k_od = mpool.tile([128, SP], BF16, tag="ko", name=f"ko{c}")
                v_od = mpool.tile([128, SP], BF16, tag="vo", name=f"vo{c}")
                nc.sync.dma_start(k_od[:, : SP - 1], k_bf[:, 1:])
                nc.sync.dma_start(v_od[:, : SP - 1], v_bf[:, 1:])
                kmaps[c], komaps[c] = k_bf, k_od
                vmaps[c], vomaps[c] = v_bf, v_od
                qflats[c] = qf

            def emit_group(c, dj, den, num, di0=0, ndi=K, nact=0):
                dje = dj - (dj % 2)
                kc = kmaps[c] if dj % 2 == 0 else komaps[c]
                vc = vmaps[c] if dj % 2 == 0 else vomaps[c]
                nm = f"{c}{dj}{di0}"
                kr = krpool.tile([128, ndi * OP], BF16, tag="kr", name=f"kr{nm}")
                tg = tpool.tile([128, ndi * OP], BF16, tag="t", name=f"t{nm}")
                eg = epool.tile([128, ndi * OP], BF16, tag="e", name=f"e{nm}")
                wg = wgpool.tile([128, ndi * OP], BF16, tag="w", name=f"w{nm}")

                k3 = kc[:].rearrange("p (h w) -> p h w", h=MR)
                kr4 = kr[:].rearrange("p (j h w) -> p j h w", j=ndi, h=HS)
                # kr_j = k_j + rel_j: per-tap add of a per-partition scalar.
                # DVE tensor_scalar runs at 4x; ACT identity+bias costs ~1
                # elem/cycle. Split taps to balance engine load.
                n_act_rel = nact
                for idx in range(ndi):
                    di = di0 + idx
                    j = di * K + dj
                    kv = k3[:, di : di + HS, dje : dje + W]
                    if idx < n_act_rel:
                        nc.scalar.activation(
                            kr4[:, idx],
                            kv,
                            mybir.ActivationFunctionType.Identity,
                            bias=rels[c][:, j : j + 1],
                        )
                    else:
                        nc.vector.tensor_scalar(
                            kr4[:, idx],
                            kv,
                            rels[c][:, j : j + 1],
                            None,
                            mybir.AluOpType.add,
                        )

                # t = kr (.) q grouped 2x tensor_tensor; q broadcast over the
                # tap dim via a stride-0 AP dim. First group: per-tap, so the
                # chain to the first PE matmul is short (pipeline fill).
                tg3 = tg[:].rearrange("p (j n) -> p j n", j=ndi)
                kr3 = kr[:].rearrange("p (j n) -> p j n", j=ndi)
                first = c == 0 and dj == 0
                eg4 = eg[:].rearrange("p (j h w) -> p j h w", j=ndi, h=HS)
                wg4 = wg[:].rearrange("p (j h w) -> p j h w", j=ndi, h=HS)
                if first:
                    for sl in ((0, 1), (1, 3), (3, 5)):
                        a, b = sl
                        qb = _strided_view(
                            qflats[c][:], 0, [(0, b - a), (1, OP)]
                        )
                        nc.vector.tensor_tensor(
                            tg3[:, a:b], kr3[:, a:b], qb, mybir.AluOpType.mult
                        )
                        nc.scalar.activation(
                            eg[:, a * OP : b * OP],
                            tg[:, a * OP : b * OP],
                            mybir.ActivationFunctionType.Exp,
                        )
                        vwin = _strided_view(
                            vc[:],
                            dje + (di0 + a) * MC,
                            [(MC, b - a), (MC, HS), (1, W)],
                        )
                        nc.vector.tensor_tensor(
                            wg4[:, a:b], eg4[:, a:b], vwin, mybir.AluOpType.mult
                        )
                else:
                    qb = _strided_view(qflats[c][:], 0, [(0, ndi), (1, OP)])
                    nc.vector.tensor_tensor(
                        tg3, kr3, qb, mybir.AluOpType.mult
                    )
                    nc.scalar.activation(
                        eg[:], tg[:], mybir.ActivationFunctionType.Exp
                    )
                    vwin = _strided_view(
                        vc[:], dje + di0 * MC, [(MC, ndi), (MC, HS), (1, W)]
                    )
                    nc.vector.tensor_tensor(
                        wg4, eg4, vwin, mybir.AluOpType.mult
                    )

                eg3 = eg[:].rearrange("p (j n) -> p j n", j=ndi)
                wg3 = wg[:].rearrange("p (j n) -> p j n", j=ndi)
                # per-tap matmuls (ISA caps one matmul at 512 out elems);
                # alternate PSUM regions so consecutive mms hit different
                # banks: h0-512, h1-512, h0-272, h1-272 per tap.
                for acc, src3 in ((den, eg3), (num, wg3)):
                    for idx in range(ndi):
                        di = di0 + idx
                        for lo, n in ((0, 512), (512, HALF - 512)):
                            for h in range(2):
                                base = h * HALF
                                nc.tensor.matmul(
                                    acc[h][:, lo : lo + n],
                                    idents[c],
                                    src3[:, idx, base + lo : base + lo + n],
                                    start=dj == 0 and di == 0,
                                    stop=dj == K - 1 and di == K - 1,
                                )

            def emit_epilogue(c, den, num):
                den_sb = opool.tile([128, OP], BF16, tag="osb", name=f"dsb{c}")
                num_sb = opool.tile([128, OP], BF16, tag="osb", name=f"nsb{c}")
                # split the PSUM->SBUF drain across both engines so the
                # PSUM banks free up fast (chunk 1's accumulators wait on
                # chunk 0's drain).
                for h in range(2):
                    base = h * HALF
                    if h == 0:
                        nc.vector.tensor_copy(
                            den_sb[:, base : base + HALF], den[h][:]
                        )
                        nc.scalar.copy(num_sb[:, base : base + HALF], num[h][:])
                    else:
                        nc.scalar.copy(
                            den_sb[:, base : base + HALF], den[h][:]
                        )
                        nc.vector.tensor_copy(
                            num_sb[:, base : base + HALF], num[h][:]
                        )
                nc.sync.dma_start(den_d[c], den_sb[:])
                nc.sync.dma_start(num_d[c], num_sb[:])

            # ---- emission schedule ----
            emit_inputs(0)
            emit_proj(0)
            emit_inputs(1)
            emit_proj(1)
            accs = []
            for c in range(NCH):
                den = [
                    apsum.tile([128, HALF], F32, tag="acc", name=f"den{c}{h}")
                    for h in range(2)
                ]
                num = [
                    apsum.tile([128, HALF], F32, tag="acc", name=f"num{c}{h}")
                    for h in range(2)
                ]
                accs.append((den, num))
                for dj in range(K):
                    nact = 2 - (dj % 2)
                    if c == 1 and dj == K - 1:
                        # split the final group so the tail drain runs on
                        # smaller quanta
                        emit_group(c, dj, den, num, 0, 3)
                        emit_group(c, dj, den, num, 3, 1)
                        emit_group(c, dj, den, num, 4, 1)
                    else:
                        emit_group(c, dj, den, num, 0, K, nact)
                    if c == 1 and dj == 0:
                        # chunk 0's epilogue emitted after chunk 1's first
                        # group: its casts wait on chunk 0's last matmuls,
                        # and emitting them earlier head-of-line blocks the
                        # in-order ACT/DVE queues.
                        emit_epilogue(0, *accs[0])
            emit_epilogue(1, *accs[1])

    nc.compile()
    _dedup_ldweights(nc)
    return nc


def _block_diag_weights(w):
    """w: (G, Cg_out, Cg_in) -> lhsT layout [NCH, 128, 128] where
    lhsT[c, ci, co] = w[g, co%32, ci%32] for matching 32-blocks."""
    out = np.zeros((NCH, 128, 128), np.float32)
    for c in range(NCH):
        for g4 in range(4):
            g = c * 4 + g4
            blk = w[g]  # (Cg_out, Cg_in)
            out[c, g4 * 32 : (g4 + 1) * 32, g4 * 32 : (g4 + 1) * 32] = blk.T
    return out


_NC_CACHE = {}


def _make_in_maps(inputs):
    x = np.asarray(inputs["x"], np.float32)
    wq = np.asarray(inputs["wq"], np.float32)
    wk = np.asarray(inputs["wk"], np.float32)
    wv = np.asarray(inputs["wv"], np.float32)
    rel_emb = np.asarray(inputs["rel_emb"], np.float32)
    q_emb = np.asarray(inputs["q_emb"], np.float32)

    bf = ml_dtypes.bfloat16
    wqb = _block_diag_weights(wq)
    wkb = _block_diag_weights(wk)
    wvb = _block_diag_weights(wv)
    idn = np.broadcast_to(np.eye(128, dtype=np.float32), (NCH, 128, 128))
    wall = np.ascontiguousarray(
        np.concatenate([wkb, wqb, wvb, idn], axis=2)
    ).astype(bf)
    relb = rel_emb.reshape(G, Cg, K * K).reshape(NCH, 128, K * K)
    qeb = q_emb.reshape(NCH, 128, 1)
    rqb = np.ascontiguousarray(np.concatenate([relb, qeb], axis=2))

    xp = np.pad(x, ((0, 0), (P, P), (P, P), (0, 0)))  # (B, 60, 60, C)

    in_maps = []
    for core in range(NCORES):
        b, half = divmod(core, 2)
        sh = xp[b, HS * half : HS * half + MR]         # (32, 60, C)
        xt = np.ascontiguousarray(sh.reshape(SP, C).T).reshape(NCH, 128, SP)
        in_maps.append(
            {
                "xt": xt.astype(bf),
                "wall": wall,
                "rqb": rqb,
            }
        )
    return in_maps


def kernel(**inputs):
    in_maps = _make_in_maps(inputs)

    if "nc" not in _NC_CACHE:
        _NC_CACHE["nc"] = build_nc()
    nc = _NC_CACHE["nc"]

    res = run_bass_kernel_spmd(nc, in_maps, core_ids=list(range(NCORES)))

    out = np.empty((B, H, W, C), np.float32)
    for core in range(NCORES):
        b, half = divmod(core, 2)
        den = res.results[core]["dd"].astype(np.float32).reshape(C, HS, W)
        num = res.results[core]["nd"].astype(np.float32).reshape(C, HS, W)
        o = num / den
        out[b, HS * half : HS * half + HS] = o.transpose(1, 2, 0)
    return out
